# revision 26
# baseline (speedup 1.0000x reference)
"""Trainium2 Bass kernel for nn_Decoder_24764781429449 (GNN message passing).

Math (per layer l of 3, N=4096 nodes, K=48 neighbors, C=128 channels, H=512):
    base   = concat([node0, zeros, edge])                  # fixed context
    mlp_in = concat([x, base])                             # [N,K,512]
    h1  = gelu(mlp_in @ W1 + b1)
    h2  = gelu(h1 @ W2 + b2)
    msg = h2 @ W3 + b3
    x   = LN1(x + sum_k(msg)/30);  x = LN2(x + dense_mlp(x));  x *= mask

Reductions:
  * W1 rows 256:384 multiply zeros -> dead.
  * x/node0 concat parts are shared across K: h1 = gelu(edge@W1d + pernode),
    pernode = x@W1a + node0@W1b + b1 (node0/b1 parts precomputed on host).
  * sum_k (h2 @ W3 + b3) = PSUM-accumulated per-k W3 matmuls, w3 pre-scaled.
  * LN1 gamma/beta folded into the dense-MLP weights for the dense path.

Distribution: data-parallel over nodes, 512 nodes/core across 8 cores.
Dtypes: edge/h1/h2/message+dense weights bf16 (full PE rate, half DMA);
residual x, pern, LN in fp32/fp32r.
Gelu runs on BOTH the Scalar (ACT table) engine and the Vector engine
(2 custom fused DVE ops: deg-9 odd poly with exact clamp tails, ~1.3e-3 abs
err), with a per-span static engine assignment to balance the two.
The per-node bias (pern) is broadcast over K either by the PE (identity
matmul accumulating into PSUM) or by a DVE scalar_tensor_tensor with a
stride-0 k-repeat, again statically assigned.
LayerNorm tail: PE transposes to row-major, bn_stats from PSUM, ACT sqrt +
DVE reciprocal for rsqrt, normalize on ACT (scale/bias ports), channel
affines in channel-major (per-partition scalars). Tail emission is
interleaved into the next half's span stream so it hides under it.
"""
import os
import numpy as np
import ml_dtypes
from contextlib import ExitStack

import concourse.bass as bass
import concourse.bacc as bacc
import concourse.tile as tile
from concourse import mybir
from concourse.bass_utils import run_bass_kernel_spmd

F32 = mybir.dt.float32
F32R = mybir.dt.float32r
BF16 = mybir.dt.bfloat16
AF = mybir.ActivationFunctionType
OP = mybir.AluOpType
BF = ml_dtypes.bfloat16

N, K, C, E, H, L = 4096, 48, 128, 128, 512, 3
NCORES = 8
NLOC = N // NCORES          # 512 nodes per core
NHALF = NLOC // 2           # 256
KPQ = 4                     # k-values per span
SPAN = KPQ * NHALF          # 1024 columns per span
NSPAN = K // KPQ            # 12 spans per half
SCALE = 30.0
EPS = 1e-5
HCH = H // C                # 4 dense hidden chunks
NRM_T = NLOC // C           # 4 row-major tiles of 128 nodes

# gelu(x) ~= x * clamp(0.5 + x*(GA + GB*u + GC*u^2 + GD*u^3 + GE*u^4), 0, 1)
# u = x^2; max abs err 1.27e-3; tails exact (clamp hits 0/1 by |x|=3.4).
GA, GB, GC, GD, GE = (0.39475106726638376, -0.060296274096807385,
                      0.00686331946932873, -0.00043311219960721437,
                      1.1276622387378065e-05)

_CACHED = {}


# ---------------- custom fused DVE gelu ops ----------------
def _register_gelu_ops():
    if "gelu_ops" in _CACHED:
        return _CACHED["gelu_ops"]
    import concourse.dve_ops as dve_ops
    from concourse.dve_spec import Spec, Src0, Src1, C0, C1, C2, sq, relu, lower
    from concourse.dve_uop import DveOpSpec
    from concourse.dve_table_gen import dve_ver_for

    u = sq(Src0)
    bodyA = (((u * Src1 + C2) * u + C1) * u + C0) * u

    def refA(in0, in1, s0, s1, imm2):
        uu = in0.astype(np.float32) ** 2
        return ((((uu * in1) + imm2) * uu + s1) * uu + s0) * uu

    m = (Src0 + C0) * Src1
    bodyB = (relu(m + C2) - relu(m - C2)) * Src1

    def refB(in0, in1, s0, s1, imm2):
        mm = (in0.astype(np.float32) + s0) * in1
        return (np.maximum(mm + imm2, 0) - np.maximum(mm - imm2, 0)) * in1

    ver = dve_ver_for("TRN2")
    ops = []
    for name, body, ref in (("GELU_P9A_ANT", bodyA, refA),
                            ("GELU_P9B_ANT", bodyB, refB)):
        if name in dve_ops._SUB_OPCODE_FOR_NAME:
            ops.append(next(o for o in dve_ops.OPS if o.name == name))
            continue
        spec = Spec(body=body, reference=ref)
        row = dve_ops._CUSTOM_DVE_ROW_BASE + len(dve_ops.OPS)
        dve_ops._SUB_OPCODE_FOR_NAME[name] = row
        op = dve_ops.DveOp(name, spec, subdim=False, uops_sha={})
        compiled = DveOpSpec(name=name, opcode=row, uops=lower(spec, ver=ver),
                             rd1_en=True)
        object.__setattr__(op, "uops_sha", {ver: compiled.sha(ver)})
        dve_ops.OPS.append(op)
        dve_ops.CUSTOM_DVE_SPECS[name] = spec
        ops.append(op)
    _CACHED["gelu_ops"] = ops
    return ops


# ---------------- packed-constant layout ----------------
class Seg:
    """Column-segment registry for the packed constant tensors."""

    def __init__(self):
        self.cols = 0
        self.off = {}

    def add(self, name, ncols):
        self.off[name] = self.cols
        self.cols += ncols
        return self.off[name]


def _seg_layout():
    # bf16 pack, ordered by DMA need-time:
    #   blkA (earliest): i128bf + pern0 + layer-0 message weights
    #   blkB: layer-0 dense weights
    #   blkC: layers 1-2 (message + dense)
    bf = Seg()
    bf.add("i128bf", C)
    bf.add("pern0", NLOC)
    bf.add("w1d0", C)
    bf.add("w20", C)
    bf.add("w3e0", C)
    for ch in range(HCH):
        bf.add(f"dw1_0_{ch}", C)
        bf.add(f"dw2_0_{ch}", C)
    for l in range(1, L):
        bf.add(f"w1d{l}", C)
        bf.add(f"w2{l}", C)
        bf.add(f"w3e{l}", C)
        for ch in range(HCH):
            bf.add(f"dw1_{l}_{ch}", C)
            bf.add(f"dw2_{l}_{ch}", C)
    # fp32r segment: everything consumed by fp32r matmuls (PE rounding rule)
    r = Seg()
    r.add("w1a1", C)
    r.add("w1a2", C)
    r.add("i128f", C)
    f = Seg()
    f.add("b3e", L)          # [C, l]
    f.add("db2", L)
    f.add("g1", L)
    f.add("b1", L)
    f.add("b2", L)
    f.add("c12", L)
    f.add("db1", L * HCH)    # [C, (l,ch)]
    f.add("maskrm", NRM_T)
    f.add("eps", 1)
    f.add("x0", NLOC)
    f.add("n0pern1", NLOC)
    f.add("n0pern2", NLOC)
    for l in range(L):
        f.add(f"gbc2_{l}", C)
        f.add(f"bbc2_{l}", C)
    return bf, f, r


def _build():
    GELU_A, GELU_B = _register_gelu_ops()
    bfseg, fseg, rseg = _seg_layout()

    # ---- per-span engine assignment (env-tunable) ----
    # A-side modes: 0 = PE-pern + ACT gelu, 1 = PE-pern + DVE 2-op gelu,
    #               2 = DVE-stt pern + ACT gelu, 3 = DVE-stt pern + DVE gelu
    # B-side modes: 0 = ACT gelu (bias port), 1 = DVE ts + 2-op gelu
    nA_dve2 = int(os.environ.get("KV_A_DVE2", "2"))
    nA_stta = int(os.environ.get("KV_A_STTA", "0"))
    nA_sttd = int(os.environ.get("KV_A_STTD", "0"))
    nB_dve = int(os.environ.get("KV_B_DVE", "4"))
    V_LAYERS = int(os.environ.get("KV_LAYERS", L))
    NORM_ACT = os.environ.get("KV_NORM_ACT", "1") == "1"
    NEWTON_IT = int(os.environ.get("KV_NR", "2"))

    amodes = [0] * NSPAN
    # spread the special spans across the 12 slots
    special = [1] * nA_dve2 + [2] * nA_stta + [3] * nA_sttd
    if special:
        step = NSPAN / len(special)
        for i, md in enumerate(special):
            amodes[min(NSPAN - 1, int(i * step + step / 2))] = md
    bmodes = [0] * NSPAN
    if nB_dve:
        step = NSPAN / nB_dve
        for i in range(nB_dve):
            bmodes[min(NSPAN - 1, int(i * step))] = 1

    nc = bacc.Bacc()

    edge_d = nc.dram_tensor("edge_km", [C, 2 * K * NHALF], BF16, kind="ExternalInput")
    packbf_d = nc.dram_tensor("packbf", [C, bfseg.cols], BF16, kind="ExternalInput")
    packf_d = nc.dram_tensor("packf", [C, fseg.cols], F32, kind="ExternalInput")
    packr_d = nc.dram_tensor("packr", [C, rseg.cols], F32R, kind="ExternalInput")
    out_d = nc.dram_tensor("out", [NLOC, C], F32, kind="ExternalOutput")

    with tile.TileContext(nc) as tc, ExitStack() as ctx:
        const = ctx.enter_context(tc.tile_pool(name="const", bufs=1))
        h1p = ctx.enter_context(tc.tile_pool(name="h1p", bufs=2))
        h2p = ctx.enter_context(tc.tile_pool(name="h2p", bufs=2))
        xbp = ctx.enter_context(tc.tile_pool(name="xbp", bufs=2))
        tbp = ctx.enter_context(tc.tile_pool(name="tbp", bufs=2))
        tl = ctx.enter_context(tc.tile_pool(name="tl", bufs=2))
        sp = ctx.enter_context(tc.tile_pool(name="sp", bufs=3, space="PSUM"))
        msump = ctx.enter_context(tc.tile_pool(name="msump", bufs=1, space="PSUM"))
        tps = ctx.enter_context(tc.tile_pool(name="tps", bufs=1, space="PSUM"))

        # ---------------- persistent SBUF ----------------
        edge = const.tile([C, 2 * K * NHALF], BF16)
        packbf = const.tile([C, bfseg.cols], BF16)
        packf = const.tile([C, fseg.cols], F32)
        packr = const.tile([C, rseg.cols], F32R)
        ebc = const.tile([C, 1], F32)
        magic = const.tile([C, 1], mybir.dt.int32)
        bbm = const.tile([C, L, NRM_T, C], F32)   # bbc2[l] * mask, per rm tile
        pern = [const.tile([C, NLOC], BF16, name=f"pern{l}") for l in range(1, L)]
        xs = [const.tile([C, NLOC], F32R, name=f"x{l}") for l in range(1, L)]

        def bfv(name, ncols=C, dt=None):
            a = packbf[:, bfseg.off[name]:bfseg.off[name] + ncols]
            return a if dt is None else a.bitcast(dt)

        def fv(name, ncols=1, dt=None):
            a = packf[:, fseg.off[name]:fseg.off[name] + ncols]
            return a if dt is None else a.bitcast(dt)

        def rv(name, ncols=1, dt=None):
            a = packr[:, rseg.off[name]:rseg.off[name] + ncols]
            return a if dt is None else a.bitcast(dt)

        nc.vector.memset(ebc, GE)
        nc.vector.memset(magic, 0x5F3759DF)
        halfc = const.tile([C, 1], F32)
        c15 = const.tile([C, 1], F32)
        negone = const.tile([C, 1], F32)
        nc.vector.memset(halfc, 0.5)
        nc.vector.memset(c15, 1.5)
        nc.vector.memset(negone, -1.0)
        bbm_done = False

        # ---------------- input DMAs ----------------
        # Measured queue behavior: the scalar (ACT) HWDGE ring moves
        # ~130 GB/s with ~3.5us start lag; the sync (SP) HWDGE ring trickles
        # (~4us/packet/engine) -- put NOTHING early on it. SWDGE (gpsimd)
        # does ~250-320 GB/s with ~5us start lag.
        #  * scalar ring: blkA (i128bf + pern0 + l0 msg weights), small
        #    vectors, fp32r pack -- the span-0 and tail-0 criticals.
        #  * gpsimd SWDGE: edge chunks in span order, with x0/n0pern,
        #    l0 dense, gbc and layer-1/2 weights slotted in by need-time.
        vcols = fseg.off["x0"]
        xcols = fseg.off["gbc2_0"]
        blkA = bfseg.off["w3e0"] + C
        blkB = bfseg.off["w1d1"]
        HC = K * NHALF                     # 12288 cols per half
        nc.scalar.dma_start(packbf[:, 0:blkA], packbf_d.ap()[:, 0:blkA])
        nc.scalar.dma_start(packf[:, 0:vcols], packf_d.ap()[:, 0:vcols])
        nc.scalar.dma_start(packr, packr_d.ap())
        g = nc.gpsimd
        g.dma_start(edge[:, 0:SPAN], edge_d.ap()[:, 0:SPAN])
        g.dma_start(edge[:, SPAN:2 * SPAN], edge_d.ap()[:, SPAN:2 * SPAN])
        g.dma_start(packf[:, vcols:xcols], packf_d.ap()[:, vcols:xcols])
        g.dma_start(packbf[:, blkA:blkB], packbf_d.ap()[:, blkA:blkB])
        g.dma_start(edge[:, 2 * SPAN:4 * SPAN], edge_d.ap()[:, 2 * SPAN:4 * SPAN])
        g.dma_start(packf[:, xcols:], packf_d.ap()[:, xcols:])
        g.dma_start(edge[:, 4 * SPAN:8 * SPAN], edge_d.ap()[:, 4 * SPAN:8 * SPAN])
        g.dma_start(edge[:, 8 * SPAN:HC], edge_d.ap()[:, 8 * SPAN:HC])
        g.dma_start(packbf[:, blkB:], packbf_d.ap()[:, blkB:])
        g.dma_start(edge[:, HC:HC + 4 * SPAN], edge_d.ap()[:, HC:HC + 4 * SPAN])
        g.dma_start(edge[:, HC + 4 * SPAN:2 * HC], edge_d.ap()[:, HC + 4 * SPAN:2 * HC])

        i128b = bfv("i128bf")
        i128f = rv("i128f", C)
        pern_all = [bfv("pern0", NLOC)] + pern
        xs_all = [fv("x0", NLOC, F32R)] + xs
        n0pern = [None, fv("n0pern1", NLOC), fv("n0pern2", NLOC)]
        w1a = [None, rv("w1a1", C), rv("w1a2", C)]
        epsc = fv("eps", 1)

        def vcol(name, l):
            return fv(name, L)[:, l:l + 1]

        for _l in range(L):
            for _t in range(NRM_T):
                nc.gpsimd.tensor_mul(
                    bbm[:, _l, _t], fv(f"bbc2_{_l}", C),
                    fv("maskrm", NRM_T)[:, _t:_t + 1].broadcast_to([C, C]))

        # ---------------- DVE gelu helper ----------------
        def emit_gelu_dve(out_ap, x_ap, n, tag):
            t = tbp.tile([C, n], F32, tag="gtb", name=f"gt_{tag}")
            nc.vector._custom_dve(GELU_A, out=t, in0=x_ap,
                                  in1=ebc.broadcast_to([C, n]),
                                  s0=GB, s1=GC, imm2=GD)
            nc.vector._custom_dve(GELU_B, out=out_ap, in0=t, in1=x_ap,
                                  s0=GA, s1=0.0, imm2=0.5)

        # ---------------- stream phase ----------------
        def emit_stream_span(l, h, s, state):
            nsl = slice(h * NHALF, (h + 1) * NHALF)
            amode, bmode = amodes[s], bmodes[s]
            col0 = h * (K * NHALF) + s * SPAN
            lw1d = bfv(f"w1d{l}")
            lw2 = bfv(f"w2{l}")
            lw3e = bfv(f"w3e{l}")

            # -- mm1: edge matmul (+ PE pern broadcast for modes 0/1) --
            # pern broadcast: bf16 identity matmul with a stride-0 k-repeat
            # moving AP (2 repeats of 256 nodes per 512-col PSUM bank) --
            # 2 MMs + 1 dedupable LDW instead of 4 fp32r MMs + 4 LDWs.
            t1 = sp.tile([C, SPAN], F32, tag="span", name="t1")
            pe_pern = amode in (0, 1)
            for j in range(2):
                jsl = slice(j * 512, (j + 1) * 512)
                nc.tensor.matmul(t1[:, jsl], lw1d, edge[:, col0 + j * 512:col0 + (j + 1) * 512],
                                 start=True, stop=not pe_pern)
            if pe_pern:
                pbc = pern_all[l][:, nsl].unsqueeze(1).broadcast_to([C, 2, NHALF])
                for j in range(2):
                    jsl = slice(j * 512, (j + 1) * 512)
                    nc.tensor.matmul(t1[:, jsl], i128b, pbc,
                                     start=False, stop=True)
            state[("t1", s)] = t1

        def emit_gelu_A(l, h, s, state):
            nsl = slice(h * NHALF, (h + 1) * NHALF)
            amode = amodes[s]
            t1 = state.pop(("t1", s))
            state[("t1d", s)] = t1   # dead after gelu-A; reused as t2
            h1 = h1p.tile([C, SPAN], BF16, tag="h1", name="h1")
            if amode in (0, 1):        # pern already in PSUM
                if amode == 0:
                    nc.scalar.activation(h1, t1, AF.Gelu)
                else:
                    emit_gelu_dve(h1, t1, SPAN, f"a{l}{h}{s}")
            else:                      # stt pern-add into SBUF then gelu
                xb = xbp.tile([C, SPAN], F32, tag="xb", name="xb")
                pbc = (pern_all[l][:, nsl]
                       .unsqueeze(1).broadcast_to([C, KPQ, NHALF]))
                nc.vector.scalar_tensor_tensor(
                    xb.rearrange("p (a b) -> p a b", a=KPQ),
                    in0=t1.rearrange("p (a b) -> p a b", a=KPQ),
                    scalar=0.0, in1=pbc, op0=OP.bypass, op1=OP.add)
                if amode == 2:
                    nc.scalar.activation(h1, xb, AF.Gelu)
                else:
                    emit_gelu_dve(h1, xb, SPAN, f"a{l}{h}{s}")
            state[("h1", s)] = h1

        def emit_mm2(l, h, s, state):
            # reuse the span's t1 PSUM tile: gelu-A has consumed it, and the
            # WAR dependency coincides with the RAW dependency on h1.
            h1 = state.pop(("h1", s))
            t2 = state.pop(("t1d", s))
            lw2 = bfv(f"w2{l}")
            for j in range(2):
                jsl = slice(j * 512, (j + 1) * 512)
                nc.tensor.matmul(t2[:, jsl], lw2, h1[:, jsl], start=True, stop=True)
            state[("t2", s)] = t2

        def emit_gelu_B(l, h, s, state):
            t2 = state.pop(("t2", s))
            h2 = h2p.tile([C, SPAN], BF16, tag="h2", name="h2")
            if bmodes[s] == 0:
                nc.scalar.activation(h2, t2, AF.Gelu, bias=vcol("b2", l))
            else:
                xb = xbp.tile([C, SPAN], F32, tag="xb", name="xb2")
                nc.vector.tensor_scalar(xb, t2, vcol("b2", l), None, op0=OP.add)
                emit_gelu_dve(h2, xb, SPAN, f"b{l}{h}{s}")
            state[("h2", s)] = h2

        def emit_msum(l, h, s, state, msum):
            h2 = state.pop(("h2", s))
            lw3e = bfv(f"w3e{l}")
            for q in range(KPQ):
                rsl = slice(q * NHALF, (q + 1) * NHALF)
                nc.tensor.matmul(msum, lw3e, h2[:, rsl],
                                 start=(s == 0 and q == 0),
                                 stop=(s == NSPAN - 1 and q == KPQ - 1))

        # ---------------- tail (generator; pumped between spans) ----------------
        def emit_tail(l, h, msum):
            nsl = slice(h * NHALF, (h + 1) * NHALF)
            last = l == V_LAYERS - 1
            # ONE PSUM bank (512 f32 cols) holds every tail intermediate via
            # sequenced region reuse (each new use is ordered after the old
            # one's last read by a WAR dependency the Tile tracker enforces):
            #   r0 [0:256]   x1rm -> xhc_ps -> dd -> x3c_ps
            #   r1 [256:512] pd rounds -> x2rm -> pp
            tailt = tps.tile([C, 2 * NHALF], F32, tag="tail", name="tailt")
            r0 = tailt[:, 0:256]
            r1 = tailt[:, 256:512]
            x1rm = r0.rearrange("p (a b) -> p a b", a=2)
            pd_r = r1
            dd = r0
            xhc_ps = r0.rearrange("p (a b) -> p a b", a=2)

            # x1 = x + msum + b3e  (channel-major, fp32)
            x1 = tl.tile([C, NHALF], F32, tag="x1")
            nc.vector.scalar_tensor_tensor(
                x1, in0=msum, scalar=vcol("b3e", l),
                in1=xs_all[l].bitcast(F32)[:, nsl], op0=OP.add, op1=OP.add)
            yield
            # transpose to row-major
            for t in range(2):
                nc.tensor.transpose(x1rm[:, t], x1[:, t * C:(t + 1) * C], i128f.bitcast(F32))
            yield

            def ln_stats(xrm, tag):
                # stats on DVE (PSUM source); the whole rsqrt chain on the
                # near-idle Pool engine via the quake bit-hack.  Pool's empty
                # FIFO matters more than per-op speed: on DVE each ~160ns
                # chain step queues behind ~1.2us stream gelus, dilating the
                # serial chain ~5x at half boundaries.
                st = tl.tile([C, 2, 6], F32, tag=f"st{tag}")
                mv = tl.tile([C, 2, 2], F32, tag=f"mv{tag}")
                for t in range(2):
                    nc.vector.bn_stats(st[:, t], xrm[:, t])
                for t in range(2):
                    nc.vector.bn_aggr(mv[:, t], st[:, t])
                var_ap = bass.AP(tensor=mv.tensor, offset=mv.offset + 1,
                                 ap=[list(mv.ap[0])] + [[2, 2]])
                mu_ap = bass.AP(tensor=mv.tensor, offset=mv.offset,
                                ap=[list(mv.ap[0])] + [[2, 2]])
                # Pool only supports tensor-tensor forms (no TensorScalarPtr),
                # so the two int quake-seed ops stay on DVE and the FP steps
                # use broadcast const tiles.
                veps = tl.tile([C, 2], F32, tag=f"ve{tag}")
                nc.gpsimd.tensor_add(veps, var_ap, epsc.broadcast_to([C, 2]))
                isd = tl.tile([C, 2], F32, tag=f"isd{tag}")
                ush = tl.tile([C, 2], mybir.dt.int32, tag=f"us{tag}")
                nc.vector.tensor_scalar(ush, veps.bitcast(mybir.dt.int32), 1, None,
                                        op0=OP.logical_shift_right)
                nc.vector.scalar_tensor_tensor(
                    isd.bitcast(mybir.dt.int32), in0=magic.broadcast_to([C, 2]),
                    scalar=0, in1=ush, op0=OP.bypass, op1=OP.subtract)
                qt = tl.tile([C, 2], F32, tag=f"qt{tag}")
                for _ in range(NEWTON_IT):
                    nc.gpsimd.tensor_mul(qt, isd, isd)
                    nc.gpsimd.tensor_mul(qt, qt, veps)
                    nc.gpsimd.tensor_mul(qt, qt, halfc.broadcast_to([C, 2]))
                    nc.gpsimd.tensor_sub(qt, c15.broadcast_to([C, 2]), qt)
                    nc.gpsimd.tensor_mul(isd, isd, qt)
                mui = tl.tile([C, 2], F32, tag=f"mui{tag}")
                nc.gpsimd.tensor_mul(mui, mu_ap, isd)
                nc.gpsimd.tensor_mul(mui, mui, negone.broadcast_to([C, 2]))
                return mv, isd, mui

            mv1, isd1, mui1 = ln_stats(x1rm, "1")
            yield
            # normalize -> xhat (rm); gamma/beta folded into dense wts
            xhat = tl.tile([C, 2, C], F32, tag="xhat")
            for t in range(2):
                if NORM_ACT:
                    nc.scalar.activation(xhat[:, t], x1rm[:, t], AF.Identity,
                                         bias=mui1[:, t:t + 1], scale=isd1[:, t:t + 1])
                else:
                    nc.vector.tensor_scalar(xhat[:, t], x1rm[:, t],
                                            mv1[:, t, 0:1], isd1[:, t:t + 1],
                                            op0=OP.subtract, op1=OP.mult)
            yield
            # transpose xhat to channel-major (f32), convert to bf16 on copy
            for t in range(2):
                nc.tensor.transpose(xhc_ps[:, t], xhat[:, t], i128f.bitcast(F32))
            xhc = tl.tile([C, 2 * C], BF16, tag="xhc")
            nc.vector.tensor_copy(xhc, xhc_ps.rearrange("p a b -> p (a b)"))
            yield
            # dense MLP: 4 hidden-chunk rounds through pd_r (each a closed
            # PSUM group), then the dd accumulation group -- the dd group
            # must not be open while pd_r groups start, since start=True
            # clears has_written for the whole 2KB zero region (the shared
            # tail bank).
            dh = tl.tile([C, HCH, NHALF], BF16, tag="dh")
            for ch in range(HCH):
                nc.tensor.matmul(pd_r, bfv(f"dw1_{l}_{ch}"), xhc, start=True, stop=True)
                nc.scalar.activation(dh[:, ch], pd_r, AF.Gelu,
                                     bias=fv("db1", L * HCH)[:, l * HCH + ch:l * HCH + ch + 1])
                if ch % 2 == 1:
                    yield
            for ch in range(HCH):
                nc.tensor.matmul(dd, bfv(f"dw2_{l}_{ch}"), dh[:, ch],
                                 start=(ch == 0), stop=(ch == HCH - 1))
            # x2 = (xhat*g1 + b1) + dd + db2   (channel-major)
            x2a = tl.tile([C, NHALF], F32, tag="x2a")
            nc.vector.tensor_scalar(x2a, xhc, vcol("g1", l), vcol("c12", l),
                                    op0=OP.mult, op1=OP.add)
            x2 = tl.tile([C, NHALF], F32, tag="x2")
            nc.vector.scalar_tensor_tensor(x2, in0=dd, scalar=0.0,
                                           in1=x2a, op0=OP.bypass, op1=OP.add)
            yield
            # LN2 (row-major); pd rounds done, reuse r1
            x2rm = r1.rearrange("p (a b) -> p a b", a=2)
            for t in range(2):
                nc.tensor.transpose(x2rm[:, t], x2[:, t * C:(t + 1) * C], i128f.bitcast(F32))
            yield
            mv2, isd2, mui2 = ln_stats(x2rm, "2")
            # fold mask into scale/bias: xhat2 = (x2 - mu)*isd*m
            isdm = tl.tile([C, 2], F32, tag="isdm")
            nc.gpsimd.tensor_mul(isdm, isd2, fv("maskrm", NRM_T)[:, 2 * h:2 * h + 2])
            muim = tl.tile([C, 2], F32, tag="muim")
            nc.gpsimd.tensor_mul(muim, mui2, fv("maskrm", NRM_T)[:, 2 * h:2 * h + 2])
            yield
            xhat2 = tl.tile([C, 2, C], F32, tag="xhat2")
            for t in range(2):
                if NORM_ACT:
                    nc.scalar.activation(xhat2[:, t], x2rm[:, t], AF.Identity,
                                         bias=muim[:, t:t + 1], scale=isdm[:, t:t + 1])
                else:
                    nc.vector.tensor_scalar(xhat2[:, t], x2rm[:, t],
                                            mv2[:, t, 0:1], isdm[:, t:t + 1],
                                            op0=OP.subtract, op1=OP.mult)
            yield
            # x3 = xhat2*gbc2 + bbm   (row-major; both ops on Pool)
            x3a = tl.tile([C, 2, C], F32, tag="x3a")
            for t in range(2):
                nc.gpsimd.tensor_mul(x3a[:, t], xhat2[:, t], fv(f"gbc2_{l}", C))
            x3 = tl.tile([C, 2, C], F32, tag="x3")
            for t in range(2):
                nc.gpsimd.tensor_add(x3[:, t], x3a[:, t], bbm[:, l, 2 * h + t])
            yield
            if last:
                for t in range(2):
                    nc.scalar.dma_start(
                        out_d.ap()[h * NHALF + t * C:h * NHALF + (t + 1) * C, :],
                        x3[:, t])
                return
            # transpose x3 back to channel-major -> xs[l+1]; compute pern[l+1]
            x3c_ps = r0.rearrange("p (a b) -> p a b", a=2)  # dd dead after x2 stt
            for t in range(2):
                nc.tensor.transpose(x3c_ps[:, t], x3[:, t], i128f.bitcast(F32))
            nc.vector.tensor_copy(xs_all[l + 1][:, nsl],
                                  x3c_ps.rearrange("p a b -> p (a b)"))
            yield
            pp = r1  # free after the LN2 normalize reads
            nc.tensor.matmul(pp, w1a[l + 1], xs_all[l + 1][:, nsl], start=True, stop=True)
            nc.vector.scalar_tensor_tensor(
                pern_all[l + 1][:, nsl], in0=pp, scalar=0.0,
                in1=n0pern[l + 1][:, nsl], op0=OP.bypass, op1=OP.add)

        # ---------------- main loop ----------------
        # Unified lag-2 pipeline over both halves: at step t the PE queue
        # gets mm1/pern for span t, gelu-A for span t-1, and mm2/gelu-B/msum
        # for span t-2.  The two-step lag means every PE consumer is emitted
        # a full span-time (~2.1us) after its gelu producer started, so a
        # gelu running long never stalls the in-order PE queue; t1 PSUM pool
        # holds 3 spans (6 banks) to cover the deeper pipeline.  Tails are
        # pumped from a deque between span steps, spilling into the next
        # half/layer's stream.
        from collections import deque
        tails = deque()

        def pump(k=1):
            for _ in range(k):
                if not tails:
                    return
                try:
                    next(tails[0])
                except StopIteration:
                    tails.popleft()

        NT = 2 * NSPAN
        for l in range(V_LAYERS):
            msumall = msump.tile([C, 2, NHALF], F32, tag="ms", name="msum")
            msums = [msumall[:, 0], msumall[:, 1]]
            states = [{}, {}]
            for t in range(NT + 2):
                if t < NT:
                    h, s = divmod(t, NSPAN)
                    if s == 0 and l > 0:
                        # the tail producing pern[l][:, h] must be fully
                        # emitted before any consumer of it is emitted
                        # (program order is dependency order for the tracker)
                        while len(tails) > (1 - h):
                            pump()
                    emit_stream_span(l, h, s, states[h])
                if 1 <= t < NT + 1:
                    ha, sa = divmod(t - 1, NSPAN)
                    emit_gelu_A(l, ha, sa, states[ha])
                if t >= 2:
                    hb, sb = divmod(t - 2, NSPAN)
                    emit_mm2(l, hb, sb, states[hb])
                    emit_gelu_B(l, hb, sb, states[hb])
                    emit_msum(l, hb, sb, states[hb], msums[hb])
                    if sb == NSPAN - 1:
                        tails.append(emit_tail(l, hb, msums[hb]))
                pump(2 if len(tails) > 1 else 1)
        while tails:
            pump()

    if os.environ.get("KV_DEDUP", "1") == "1":
        _dedup_ldweights(nc)
    nc.compile()
    return nc


def _dedup_ldweights(nc):
    """Drop InstLdweights that reload the PE stationary operand already in
    the array. tile_legalize emits one LDW per bf16 matmul even for runs of
    matmuls sharing one weight (mm1 x2, pern x2, msum x4); the redundant LDW
    serializes ~107ns each on the PE queue. fp32/fp32r matmuls (transposes,
    tail pern-next) are self-loading at walrus level and invalidate the
    loaded weight, as does any kept LDW with a different key."""
    removed = 0
    for blk in nc.main_func.blocks:
        last_key = None
        keep = []
        for inst in blk.instructions:
            tn = type(inst).__name__
            if tn == "InstLdweights":
                w = inst.ins[0]
                si = inst.sync_info
                clean = si is None or (not si.on_wait and not si.on_update)
                key = (str(w.memref), w.offset, str(w.ap), str(w.dtype),
                       str(inst.perf_mode), str(inst.is_transpose),
                       str(inst.tile_position))
                if clean and str(w.dtype) == "dt.bfloat16" and key == last_key:
                    removed += 1
                    continue          # weights already loaded; drop
                last_key = key
            elif tn == "InstMatmult":
                wd = str(inst.ins[-1].dtype)
                if wd not in ("dt.bfloat16",):
                    last_key = None   # walrus inserts its own LDW here
            keep.append(inst)
        blk.instructions[:] = keep
    return removed


def _prep_inputs(inputs):
    """Host-side: shard over nodes, relayout, fold weight-only arithmetic."""
    bfseg, fseg, rseg = _seg_layout()
    nf = np.asarray(inputs["node_features"], dtype=np.float32)
    ef = np.asarray(inputs["edge_features"], dtype=np.float32)
    mask = np.asarray(inputs["mask"], dtype=np.float32)
    w1 = np.asarray(inputs["msg_w1"], dtype=np.float32)
    w2 = np.asarray(inputs["msg_w2"], dtype=np.float32)
    w3 = np.asarray(inputs["msg_w3"], dtype=np.float32)
    b1 = np.asarray(inputs["msg_b1"], dtype=np.float32)
    b2 = np.asarray(inputs["msg_b2"], dtype=np.float32)
    b3 = np.asarray(inputs["msg_b3"], dtype=np.float32)
    dw1 = np.asarray(inputs["d_w1"], dtype=np.float32)
    db1 = np.asarray(inputs["d_b1"], dtype=np.float32)
    dw2 = np.asarray(inputs["d_w2"], dtype=np.float32)
    db2 = np.asarray(inputs["d_b2"], dtype=np.float32)
    g1 = np.asarray(inputs["ln1_g"], dtype=np.float32)
    bb1 = np.asarray(inputs["ln1_b"], dtype=np.float32)
    g2 = np.asarray(inputs["ln2_g"], dtype=np.float32)
    bb2 = np.asarray(inputs["ln2_b"], dtype=np.float32)

    w1a = w1[:, 0:C, :]
    w1b = w1[:, C:2 * C, :]
    w1d = w1[:, 3 * C:4 * C, :]
    w3e = w3 / SCALE
    b3e = b3 * (K / SCALE)
    # dense folds: input is xhat (normalized, no affine); LN1 g/b folded in.
    dw1f = g1[:, :, None] * dw1                     # [L, C, H]
    db1f = db1 + np.einsum("lc,lch->lh", bb1, dw1)  # [L, H]

    # ---- shared packed tensors ----
    packbf_shared = np.zeros((C, bfseg.cols), dtype=BF)
    for l in range(L):
        o = bfseg.off[f"w1d{l}"]
        packbf_shared[:, o:o + C] = w1d[l].astype(BF)
        o = bfseg.off[f"w2{l}"]
        packbf_shared[:, o:o + C] = w2[l].astype(BF)
        o = bfseg.off[f"w3e{l}"]
        packbf_shared[:, o:o + C] = w3e[l].astype(BF)
        for ch in range(HCH):
            o = bfseg.off[f"dw1_{l}_{ch}"]
            packbf_shared[:, o:o + C] = dw1f[l][:, ch * C:(ch + 1) * C].astype(BF)
            o = bfseg.off[f"dw2_{l}_{ch}"]
            packbf_shared[:, o:o + C] = dw2[l][ch * C:(ch + 1) * C, :].astype(BF)
    o = bfseg.off["i128bf"]
    packbf_shared[:, o:o + C] = np.eye(C, dtype=BF)

    packf_shared = np.zeros((C, fseg.cols), dtype=np.float32)
    packr_shared = np.zeros((C, rseg.cols), dtype=np.float32)

    def put(name, arr):
        o = fseg.off[name]
        packf_shared[:, o:o + arr.shape[1]] = arr

    def putr(name, arr):
        o = rseg.off[name]
        packr_shared[:, o:o + arr.shape[1]] = arr

    putr("w1a1", w1a[1])
    putr("w1a2", w1a[2])
    putr("i128f", np.eye(C, dtype=np.float32))
    for l in range(L):
        put(f"gbc2_{l}", np.broadcast_to(g2[l][None, :], (C, C)))
        put(f"bbc2_{l}", np.broadcast_to(bb2[l][None, :], (C, C)))
    put("b3e", b3e.T.copy())      # [C, L]
    put("db2", db2.T.copy())
    put("g1", g1.T.copy())
    put("b1", bb1.T.copy())
    put("b2", b2.T.copy())
    put("c12", (bb1 + db2).T.copy())
    put("db1", db1f.reshape(L * HCH, C).T.copy())
    packf_shared[:, fseg.off["eps"]] = EPS

    # host-computed per-node biases
    pern0_full = nf @ (w1a[0] + w1b[0]) + b1[0]          # [N, C]
    n0p1_full = nf @ w1b[1] + b1[1]
    n0p2_full = nf @ w1b[2] + b1[2]

    in_maps = []
    for core in range(NCORES):
        n0 = core * NLOC
        esh = ef[n0:n0 + NLOC]                       # [512, 48, 128]
        ekm = esh.transpose(2, 1, 0)                 # [128c, 48k, 512n]
        ekm = ekm.reshape(C, K, 2, NHALF).transpose(0, 2, 1, 3)
        ekm = np.ascontiguousarray(ekm.reshape(C, 2 * K * NHALF)).astype(BF)
        packf = packf_shared.copy()
        packbf = packbf_shared.copy()
        o = fseg.off["x0"]
        packf[:, o:o + NLOC] = nf[n0:n0 + NLOC].T
        o = bfseg.off["pern0"]
        packbf[:, o:o + NLOC] = pern0_full[n0:n0 + NLOC].T.astype(BF)
        o = fseg.off["n0pern1"]
        packf[:, o:o + NLOC] = n0p1_full[n0:n0 + NLOC].T
        o = fseg.off["n0pern2"]
        packf[:, o:o + NLOC] = n0p2_full[n0:n0 + NLOC].T
        o = fseg.off["maskrm"]
        packf[:, o:o + NRM_T] = mask[n0:n0 + NLOC].reshape(NRM_T, C).T
        in_maps.append({"edge_km": ekm, "packbf": packbf, "packf": packf,
                        "packr": packr_shared})
    return in_maps


def kernel(**inputs) -> np.ndarray:
    if "nc" not in _CACHED:
        _CACHED["nc"] = _build()
    nc = _CACHED["nc"]
    in_maps = _prep_inputs(inputs)
    res = run_bass_kernel_spmd(nc, in_maps, core_ids=list(range(NCORES)))
    out = np.concatenate([res.results[c]["out"] for c in range(NCORES)], axis=0)
    _CACHED["last_results"] = res
    return out



# revision 28
# speedup vs baseline: 1.1445x; 1.1445x over previous
"""Trainium2 Bass kernel for nn_Decoder_24764781429449 (GNN message passing).

Math (per layer l of 3, N=4096 nodes, K=48 neighbors, C=128 channels, H=512):
    base   = concat([node0, zeros, edge])                  # fixed context
    mlp_in = concat([x, base])                             # [N,K,512]
    h1  = gelu(mlp_in @ W1 + b1)
    h2  = gelu(h1 @ W2 + b2)
    msg = h2 @ W3 + b3
    x   = LN1(x + sum_k(msg)/30);  x = LN2(x + dense_mlp(x));  x *= mask

Reductions:
  * W1 rows 256:384 multiply zeros -> dead.
  * x/node0 concat parts are shared across K: h1 = gelu(edge@W1d + pernode),
    pernode = x@W1a + node0@W1b + b1 (node0/b1 parts precomputed on host).
  * sum_k (h2 @ W3 + b3) = PSUM-accumulated per-k W3 matmuls, w3 pre-scaled.
  * LN1 gamma/beta folded into the dense-MLP weights for the dense path.

Distribution: data-parallel over nodes, 512 nodes/core across 8 cores.
Dtypes: edge/h1/h2/message+dense weights bf16 (full PE rate, half DMA);
residual x, pern, LN in fp32/fp32r.
Gelu runs on BOTH the Scalar (ACT table) engine and the Vector engine
(2 custom fused DVE ops: deg-9 odd poly with exact clamp tails, ~1.3e-3 abs
err), with a per-span static engine assignment to balance the two.
The per-node bias (pern) is broadcast over K either by the PE (identity
matmul accumulating into PSUM) or by a DVE scalar_tensor_tensor with a
stride-0 k-repeat, again statically assigned.
LayerNorm tail: PE transposes to row-major, bn_stats from PSUM, ACT sqrt +
DVE reciprocal for rsqrt, normalize on ACT (scale/bias ports), channel
affines in channel-major (per-partition scalars). Tail emission is
interleaved into the next half's span stream so it hides under it.
"""
import os
import numpy as np
import ml_dtypes
from contextlib import ExitStack

import concourse.bass as bass
import concourse.bacc as bacc
import concourse.tile as tile
from concourse import mybir
from concourse.bass_utils import run_bass_kernel_spmd

F32 = mybir.dt.float32
F32R = mybir.dt.float32r
BF16 = mybir.dt.bfloat16
AF = mybir.ActivationFunctionType
OP = mybir.AluOpType
BF = ml_dtypes.bfloat16

N, K, C, E, H, L = 4096, 48, 128, 128, 512, 3
NCORES = 8
NLOC = N // NCORES          # 512 nodes per core
NHALF = NLOC // 2           # 256
KPQ = 4                     # k-values per span
SPAN = KPQ * NHALF          # 1024 columns per span
NSPAN = K // KPQ            # 12 spans per half
SCALE = 30.0
EPS = 1e-5
HCH = H // C                # 4 dense hidden chunks
NRM_T = NLOC // C           # 4 row-major tiles of 128 nodes

# gelu(x) ~= x * clamp(0.5 + x*(GA + GB*u + GC*u^2 + GD*u^3 + GE*u^4), 0, 1)
# u = x^2; max abs err 1.27e-3; tails exact (clamp hits 0/1 by |x|=3.4).
GA, GB, GC, GD, GE = (0.39475106726638376, -0.060296274096807385,
                      0.00686331946932873, -0.00043311219960721437,
                      1.1276622387378065e-05)

_CACHED = {}


# ---------------- custom fused DVE gelu ops ----------------
def _register_gelu_ops():
    if "gelu_ops" in _CACHED:
        return _CACHED["gelu_ops"]
    import concourse.dve_ops as dve_ops
    from concourse.dve_spec import Spec, Src0, Src1, C0, C1, C2, sq, relu, lower
    from concourse.dve_uop import DveOpSpec
    from concourse.dve_table_gen import dve_ver_for

    u = sq(Src0)
    bodyA = (((u * Src1 + C2) * u + C1) * u + C0) * u

    def refA(in0, in1, s0, s1, imm2):
        uu = in0.astype(np.float32) ** 2
        return ((((uu * in1) + imm2) * uu + s1) * uu + s0) * uu

    m = (Src0 + C0) * Src1
    bodyB = (relu(m + C2) - relu(m - C2)) * Src1

    def refB(in0, in1, s0, s1, imm2):
        mm = (in0.astype(np.float32) + s0) * in1
        return (np.maximum(mm + imm2, 0) - np.maximum(mm - imm2, 0)) * in1

    ver = dve_ver_for("TRN2")
    ops = []
    for name, body, ref in (("GELU_P9A_ANT", bodyA, refA),
                            ("GELU_P9B_ANT", bodyB, refB)):
        if name in dve_ops._SUB_OPCODE_FOR_NAME:
            ops.append(next(o for o in dve_ops.OPS if o.name == name))
            continue
        spec = Spec(body=body, reference=ref)
        row = dve_ops._CUSTOM_DVE_ROW_BASE + len(dve_ops.OPS)
        dve_ops._SUB_OPCODE_FOR_NAME[name] = row
        op = dve_ops.DveOp(name, spec, subdim=False, uops_sha={})
        compiled = DveOpSpec(name=name, opcode=row, uops=lower(spec, ver=ver),
                             rd1_en=True)
        object.__setattr__(op, "uops_sha", {ver: compiled.sha(ver)})
        dve_ops.OPS.append(op)
        dve_ops.CUSTOM_DVE_SPECS[name] = spec
        ops.append(op)
    _CACHED["gelu_ops"] = ops
    return ops


# ---------------- packed-constant layout ----------------
class Seg:
    """Column-segment registry for the packed constant tensors."""

    def __init__(self):
        self.cols = 0
        self.off = {}

    def add(self, name, ncols):
        self.off[name] = self.cols
        self.cols += ncols
        return self.off[name]


def _seg_layout():
    # bf16 pack, ordered by DMA need-time:
    #   blkA (earliest): i128bf + pern0 + layer-0 message weights
    #   blkB: layer-0 dense weights
    #   blkC: layers 1-2 (message + dense)
    bf = Seg()
    bf.add("i128bf", C)
    bf.add("pern0", NLOC)
    bf.add("w1d0", C)
    bf.add("w20", C)
    bf.add("w3e0", C)
    for ch in range(HCH):
        bf.add(f"dw1_0_{ch}", C)
        bf.add(f"dw2_0_{ch}", C)
    for l in range(1, L):
        bf.add(f"w1d{l}", C)
        bf.add(f"w2{l}", C)
        bf.add(f"w3e{l}", C)
        for ch in range(HCH):
            bf.add(f"dw1_{l}_{ch}", C)
            bf.add(f"dw2_{l}_{ch}", C)
    # fp32r segment: everything consumed by fp32r matmuls (PE rounding rule)
    r = Seg()
    r.add("w1a1", C)
    r.add("w1a2", C)
    r.add("i128f", C)
    f = Seg()
    f.add("b3e", L)          # [C, l]
    f.add("db2", L)
    f.add("g1", L)
    f.add("b1", L)
    f.add("b2", L)
    f.add("c12", L)
    f.add("db1", L * HCH)    # [C, (l,ch)]
    f.add("maskrm", NRM_T)
    f.add("eps", 1)
    f.add("x0", NLOC)
    f.add("n0pern1", NLOC)
    f.add("n0pern2", NLOC)
    for l in range(L):
        f.add(f"gbc2_{l}", C)
        f.add(f"bbc2_{l}", C)
    return bf, f, r


def _build():
    GELU_A, GELU_B = _register_gelu_ops()
    bfseg, fseg, rseg = _seg_layout()

    # ---- per-span engine assignment (env-tunable) ----
    # A-side modes: 0 = PE-pern + ACT gelu, 1 = PE-pern + DVE 2-op gelu,
    #               2 = DVE-stt pern + ACT gelu, 3 = DVE-stt pern + DVE gelu
    # B-side modes: 0 = ACT gelu (bias port), 1 = DVE ts + 2-op gelu
    nA_dve2 = int(os.environ.get("KV_A_DVE2", "2"))
    nA_stta = int(os.environ.get("KV_A_STTA", "0"))
    nA_sttd = int(os.environ.get("KV_A_STTD", "0"))
    nB_dve = int(os.environ.get("KV_B_DVE", "4"))
    V_LAYERS = int(os.environ.get("KV_LAYERS", L))
    NORM_ACT = os.environ.get("KV_NORM_ACT", "1") == "1"
    NEWTON_IT = int(os.environ.get("KV_NR", "2"))

    amodes = [0] * NSPAN
    # spread the special spans across the 12 slots
    special = [1] * nA_dve2 + [2] * nA_stta + [3] * nA_sttd
    if special:
        step = NSPAN / len(special)
        for i, md in enumerate(special):
            amodes[min(NSPAN - 1, int(i * step + step / 2))] = md
    bmodes = [0] * NSPAN
    if nB_dve:
        step = NSPAN / nB_dve
        for i in range(nB_dve):
            bmodes[min(NSPAN - 1, int(i * step))] = 1

    nc = bacc.Bacc()

    edge_d = nc.dram_tensor("edge_km", [C, 2 * K * NHALF], BF16, kind="ExternalInput")
    packbf_d = nc.dram_tensor("packbf", [C, bfseg.cols], BF16, kind="ExternalInput")
    packf_d = nc.dram_tensor("packf", [C, fseg.cols], F32, kind="ExternalInput")
    packr_d = nc.dram_tensor("packr", [C, rseg.cols], F32R, kind="ExternalInput")
    out_d = nc.dram_tensor("out", [NLOC, C], F32, kind="ExternalOutput")

    with tile.TileContext(nc) as tc, ExitStack() as ctx:
        const = ctx.enter_context(tc.tile_pool(name="const", bufs=1))
        h1p = ctx.enter_context(tc.tile_pool(name="h1p", bufs=2))
        h2p = ctx.enter_context(tc.tile_pool(name="h2p", bufs=2))
        xbp = ctx.enter_context(tc.tile_pool(name="xbp", bufs=2))
        tbp = ctx.enter_context(tc.tile_pool(name="tbp", bufs=2))
        tl = ctx.enter_context(tc.tile_pool(name="tl", bufs=2))
        sp = ctx.enter_context(tc.tile_pool(name="sp", bufs=3, space="PSUM"))
        msump = ctx.enter_context(tc.tile_pool(name="msump", bufs=1, space="PSUM"))
        tps = ctx.enter_context(tc.tile_pool(name="tps", bufs=1, space="PSUM"))

        # ---------------- persistent SBUF ----------------
        edge = const.tile([C, 2 * K * NHALF], BF16)
        packbf = const.tile([C, bfseg.cols], BF16)
        packf = const.tile([C, fseg.cols], F32)
        packr = const.tile([C, rseg.cols], F32R)
        ebc = const.tile([C, 1], F32)
        magic = const.tile([C, 1], mybir.dt.int32)
        bbm = const.tile([C, L, NRM_T, C], F32)   # bbc2[l] * mask, per rm tile
        pern = [const.tile([C, NLOC], BF16, name=f"pern{l}") for l in range(1, L)]
        xs = [const.tile([C, NLOC], F32R, name=f"x{l}") for l in range(1, L)]

        def bfv(name, ncols=C, dt=None):
            a = packbf[:, bfseg.off[name]:bfseg.off[name] + ncols]
            return a if dt is None else a.bitcast(dt)

        def fv(name, ncols=1, dt=None):
            a = packf[:, fseg.off[name]:fseg.off[name] + ncols]
            return a if dt is None else a.bitcast(dt)

        def rv(name, ncols=1, dt=None):
            a = packr[:, rseg.off[name]:rseg.off[name] + ncols]
            return a if dt is None else a.bitcast(dt)

        nc.vector.memset(ebc, GE)
        nc.vector.memset(magic, 0x5F3759DF)
        bbm_done = False

        # ---------------- input DMAs ----------------
        # Measured queue behavior: the scalar (ACT) HWDGE ring moves
        # ~130 GB/s with ~3.5us start lag; the sync (SP) HWDGE ring trickles
        # (~4us/packet/engine) -- put NOTHING early on it. SWDGE (gpsimd)
        # does ~250-320 GB/s with ~5us start lag.
        #  * scalar ring: blkA (i128bf + pern0 + l0 msg weights), small
        #    vectors, fp32r pack -- the span-0 and tail-0 criticals.
        #  * gpsimd SWDGE: edge chunks in span order, with x0/n0pern,
        #    l0 dense, gbc and layer-1/2 weights slotted in by need-time.
        vcols = fseg.off["x0"]
        xcols = fseg.off["gbc2_0"]
        blkA = bfseg.off["w3e0"] + C
        blkB = bfseg.off["w1d1"]
        HC = K * NHALF                     # 12288 cols per half
        nc.scalar.dma_start(packbf[:, 0:blkA], packbf_d.ap()[:, 0:blkA])
        nc.scalar.dma_start(packf[:, 0:vcols], packf_d.ap()[:, 0:vcols])
        nc.scalar.dma_start(packr, packr_d.ap())
        g = nc.gpsimd
        g.dma_start(edge[:, 0:SPAN], edge_d.ap()[:, 0:SPAN])
        g.dma_start(edge[:, SPAN:2 * SPAN], edge_d.ap()[:, SPAN:2 * SPAN])
        g.dma_start(packf[:, vcols:xcols], packf_d.ap()[:, vcols:xcols])
        g.dma_start(packbf[:, blkA:blkB], packbf_d.ap()[:, blkA:blkB])
        g.dma_start(edge[:, 2 * SPAN:4 * SPAN], edge_d.ap()[:, 2 * SPAN:4 * SPAN])
        g.dma_start(packf[:, xcols:], packf_d.ap()[:, xcols:])
        g.dma_start(edge[:, 4 * SPAN:8 * SPAN], edge_d.ap()[:, 4 * SPAN:8 * SPAN])
        g.dma_start(edge[:, 8 * SPAN:HC], edge_d.ap()[:, 8 * SPAN:HC])
        g.dma_start(packbf[:, blkB:], packbf_d.ap()[:, blkB:])
        g.dma_start(edge[:, HC:HC + 4 * SPAN], edge_d.ap()[:, HC:HC + 4 * SPAN])
        g.dma_start(edge[:, HC + 4 * SPAN:2 * HC], edge_d.ap()[:, HC + 4 * SPAN:2 * HC])

        i128b = bfv("i128bf")
        i128f = rv("i128f", C)
        pern_all = [bfv("pern0", NLOC)] + pern
        xs_all = [fv("x0", NLOC, F32R)] + xs
        n0pern = [None, fv("n0pern1", NLOC), fv("n0pern2", NLOC)]
        w1a = [None, rv("w1a1", C), rv("w1a2", C)]
        epsc = fv("eps", 1)

        def vcol(name, l):
            return fv(name, L)[:, l:l + 1]

        for _l in range(L):
            for _t in range(NRM_T):
                nc.gpsimd.tensor_mul(
                    bbm[:, _l, _t], fv(f"bbc2_{_l}", C),
                    fv("maskrm", NRM_T)[:, _t:_t + 1].broadcast_to([C, C]))

        # ---------------- DVE gelu helper ----------------
        def emit_gelu_dve(out_ap, x_ap, n, tag):
            t = tbp.tile([C, n], F32, tag="gtb", name=f"gt_{tag}")
            nc.vector._custom_dve(GELU_A, out=t, in0=x_ap,
                                  in1=ebc.broadcast_to([C, n]),
                                  s0=GB, s1=GC, imm2=GD)
            nc.vector._custom_dve(GELU_B, out=out_ap, in0=t, in1=x_ap,
                                  s0=GA, s1=0.0, imm2=0.5)

        # ---------------- stream phase ----------------
        def emit_stream_span(l, h, s, state):
            nsl = slice(h * NHALF, (h + 1) * NHALF)
            amode, bmode = amodes[s], bmodes[s]
            col0 = h * (K * NHALF) + s * SPAN
            lw1d = bfv(f"w1d{l}")
            lw2 = bfv(f"w2{l}")
            lw3e = bfv(f"w3e{l}")

            # -- mm1: edge matmul (+ PE pern broadcast for modes 0/1) --
            # pern broadcast: bf16 identity matmul with a stride-0 k-repeat
            # moving AP (2 repeats of 256 nodes per 512-col PSUM bank) --
            # 2 MMs + 1 dedupable LDW instead of 4 fp32r MMs + 4 LDWs.
            t1 = sp.tile([C, SPAN], F32, tag="span", name="t1")
            pe_pern = amode in (0, 1)
            for j in range(2):
                jsl = slice(j * 512, (j + 1) * 512)
                nc.tensor.matmul(t1[:, jsl], lw1d, edge[:, col0 + j * 512:col0 + (j + 1) * 512],
                                 start=True, stop=not pe_pern)
            if pe_pern:
                pbc = pern_all[l][:, nsl].unsqueeze(1).broadcast_to([C, 2, NHALF])
                for j in range(2):
                    jsl = slice(j * 512, (j + 1) * 512)
                    nc.tensor.matmul(t1[:, jsl], i128b, pbc,
                                     start=False, stop=True)
            state[("t1", s)] = t1

        def emit_gelu_A(l, h, s, state):
            nsl = slice(h * NHALF, (h + 1) * NHALF)
            amode = amodes[s]
            t1 = state.pop(("t1", s))
            state[("t1d", s)] = t1   # dead after gelu-A; reused as t2
            h1 = h1p.tile([C, SPAN], BF16, tag="h1", name="h1")
            if amode in (0, 1):        # pern already in PSUM
                if amode == 0:
                    nc.scalar.activation(h1, t1, AF.Gelu)
                else:
                    emit_gelu_dve(h1, t1, SPAN, f"a{l}{h}{s}")
            else:                      # stt pern-add into SBUF then gelu
                xb = xbp.tile([C, SPAN], F32, tag="xb", name="xb")
                pbc = (pern_all[l][:, nsl]
                       .unsqueeze(1).broadcast_to([C, KPQ, NHALF]))
                nc.vector.scalar_tensor_tensor(
                    xb.rearrange("p (a b) -> p a b", a=KPQ),
                    in0=t1.rearrange("p (a b) -> p a b", a=KPQ),
                    scalar=0.0, in1=pbc, op0=OP.bypass, op1=OP.add)
                if amode == 2:
                    nc.scalar.activation(h1, xb, AF.Gelu)
                else:
                    emit_gelu_dve(h1, xb, SPAN, f"a{l}{h}{s}")
            state[("h1", s)] = h1

        def emit_mm2(l, h, s, state):
            # reuse the span's t1 PSUM tile: gelu-A has consumed it, and the
            # WAR dependency coincides with the RAW dependency on h1.
            h1 = state.pop(("h1", s))
            t2 = state.pop(("t1d", s))
            lw2 = bfv(f"w2{l}")
            for j in range(2):
                jsl = slice(j * 512, (j + 1) * 512)
                nc.tensor.matmul(t2[:, jsl], lw2, h1[:, jsl], start=True, stop=True)
            state[("t2", s)] = t2

        def emit_gelu_B(l, h, s, state):
            t2 = state.pop(("t2", s))
            h2 = h2p.tile([C, SPAN], BF16, tag="h2", name="h2")
            if bmodes[s] == 0:
                nc.scalar.activation(h2, t2, AF.Gelu, bias=vcol("b2", l))
            else:
                xb = xbp.tile([C, SPAN], F32, tag="xb", name="xb2")
                nc.vector.tensor_scalar(xb, t2, vcol("b2", l), None, op0=OP.add)
                emit_gelu_dve(h2, xb, SPAN, f"b{l}{h}{s}")
            state[("h2", s)] = h2

        def emit_msum(l, h, s, state, msum):
            h2 = state.pop(("h2", s))
            lw3e = bfv(f"w3e{l}")
            for q in range(KPQ):
                rsl = slice(q * NHALF, (q + 1) * NHALF)
                nc.tensor.matmul(msum, lw3e, h2[:, rsl],
                                 start=(s == 0 and q == 0),
                                 stop=(s == NSPAN - 1 and q == KPQ - 1))

        # ---------------- tail (generator; pumped between spans) ----------------
        def emit_tail(l, h, msum):
            nsl = slice(h * NHALF, (h + 1) * NHALF)
            last = l == V_LAYERS - 1
            # ONE PSUM bank (512 f32 cols) holds every tail intermediate via
            # sequenced region reuse (each new use is ordered after the old
            # one's last read by a WAR dependency the Tile tracker enforces):
            #   r0 [0:256]   x1rm -> xhc_ps -> dd -> x3c_ps
            #   r1 [256:512] pd rounds -> x2rm -> pp
            tailt = tps.tile([C, 2 * NHALF], F32, tag="tail", name="tailt")
            r0 = tailt[:, 0:256]
            r1 = tailt[:, 256:512]
            x1rm = r0.rearrange("p (a b) -> p a b", a=2)
            pd_r = r1
            dd = r0
            xhc_ps = r0.rearrange("p (a b) -> p a b", a=2)

            # x1 = x + msum + b3e  (channel-major, fp32)
            x1 = tl.tile([C, NHALF], F32, tag="x1")
            nc.vector.scalar_tensor_tensor(
                x1, in0=msum, scalar=vcol("b3e", l),
                in1=xs_all[l].bitcast(F32)[:, nsl], op0=OP.add, op1=OP.add)
            yield
            # transpose to row-major
            for t in range(2):
                nc.tensor.transpose(x1rm[:, t], x1[:, t * C:(t + 1) * C], i128f.bitcast(F32))
            yield

            def ln_stats(xrm, tag):
                # stats on DVE (PSUM source); the whole rsqrt chain on the
                # near-idle Pool engine via the quake bit-hack.  Pool's empty
                # FIFO matters more than per-op speed: on DVE each ~160ns
                # chain step queues behind ~1.2us stream gelus, dilating the
                # serial chain ~5x at half boundaries.
                st = tl.tile([C, 2, 6], F32, tag=f"st{tag}")
                mv = tl.tile([C, 2, 2], F32, tag=f"mv{tag}")
                for t in range(2):
                    nc.vector.bn_stats(st[:, t], xrm[:, t])
                for t in range(2):
                    nc.vector.bn_aggr(mv[:, t], st[:, t])
                var_ap = bass.AP(tensor=mv.tensor, offset=mv.offset + 1,
                                 ap=[list(mv.ap[0])] + [[2, 2]])
                mu_ap = bass.AP(tensor=mv.tensor, offset=mv.offset,
                                ap=[list(mv.ap[0])] + [[2, 2]])
                veps = tl.tile([C, 2], F32, tag=f"ve{tag}")
                nc.vector.tensor_scalar(veps, var_ap, EPS, None, op0=OP.add)
                isd = tl.tile([C, 2], F32, tag=f"isd{tag}")
                ush = tl.tile([C, 2], mybir.dt.int32, tag=f"us{tag}")
                nc.vector.tensor_scalar(ush, veps.bitcast(mybir.dt.int32), 1, None,
                                        op0=OP.logical_shift_right)
                nc.vector.scalar_tensor_tensor(
                    isd.bitcast(mybir.dt.int32), in0=magic.broadcast_to([C, 2]),
                    scalar=0, in1=ush, op0=OP.bypass, op1=OP.subtract)
                qt = tl.tile([C, 2], F32, tag=f"qt{tag}")
                for _ in range(NEWTON_IT):
                    nc.vector.tensor_mul(qt, isd, isd)
                    nc.vector.tensor_mul(qt, qt, veps)
                    nc.vector.tensor_scalar(qt, qt, -0.5, 1.5, op0=OP.mult, op1=OP.add)
                    nc.vector.tensor_mul(isd, isd, qt)
                mui = tl.tile([C, 2], F32, tag=f"mui{tag}")
                nc.vector.scalar_tensor_tensor(mui, in0=mu_ap, scalar=-1.0,
                                               in1=isd, op0=OP.mult, op1=OP.mult)
                return mv, isd, mui

            mv1, isd1, mui1 = ln_stats(x1rm, "1")
            yield
            # normalize -> xhat (rm); gamma/beta folded into dense wts
            xhat = tl.tile([C, 2, C], F32, tag="xhat")
            for t in range(2):
                if NORM_ACT:
                    nc.scalar.activation(xhat[:, t], x1rm[:, t], AF.Identity,
                                         bias=mui1[:, t:t + 1], scale=isd1[:, t:t + 1])
                else:
                    nc.vector.tensor_scalar(xhat[:, t], x1rm[:, t],
                                            mv1[:, t, 0:1], isd1[:, t:t + 1],
                                            op0=OP.subtract, op1=OP.mult)
            yield
            # transpose xhat to channel-major (f32), convert to bf16 on copy
            for t in range(2):
                nc.tensor.transpose(xhc_ps[:, t], xhat[:, t], i128f.bitcast(F32))
            xhc = tl.tile([C, 2 * C], BF16, tag="xhc")
            nc.vector.tensor_copy(xhc, xhc_ps.rearrange("p a b -> p (a b)"))
            yield
            # dense MLP: 4 hidden-chunk rounds through pd_r (each a closed
            # PSUM group), then the dd accumulation group -- the dd group
            # must not be open while pd_r groups start, since start=True
            # clears has_written for the whole 2KB zero region (the shared
            # tail bank).
            dh = tl.tile([C, HCH, NHALF], BF16, tag="dh")
            for ch in range(HCH):
                nc.tensor.matmul(pd_r, bfv(f"dw1_{l}_{ch}"), xhc, start=True, stop=True)
                nc.scalar.activation(dh[:, ch], pd_r, AF.Gelu,
                                     bias=fv("db1", L * HCH)[:, l * HCH + ch:l * HCH + ch + 1])
                if ch % 2 == 1:
                    yield
            for ch in range(HCH):
                nc.tensor.matmul(dd, bfv(f"dw2_{l}_{ch}"), dh[:, ch],
                                 start=(ch == 0), stop=(ch == HCH - 1))
            # x2 = (xhat*g1 + b1) + dd + db2   (channel-major)
            x2a = tl.tile([C, NHALF], F32, tag="x2a")
            nc.vector.tensor_scalar(x2a, xhc, vcol("g1", l), vcol("c12", l),
                                    op0=OP.mult, op1=OP.add)
            x2 = tl.tile([C, NHALF], F32, tag="x2")
            nc.vector.scalar_tensor_tensor(x2, in0=dd, scalar=0.0,
                                           in1=x2a, op0=OP.bypass, op1=OP.add)
            yield
            # LN2 (row-major); pd rounds done, reuse r1
            x2rm = r1.rearrange("p (a b) -> p a b", a=2)
            for t in range(2):
                nc.tensor.transpose(x2rm[:, t], x2[:, t * C:(t + 1) * C], i128f.bitcast(F32))
            yield
            mv2, isd2, mui2 = ln_stats(x2rm, "2")
            # fold mask into scale/bias: xhat2 = (x2 - mu)*isd*m
            isdm = tl.tile([C, 2], F32, tag="isdm")
            nc.gpsimd.tensor_mul(isdm, isd2, fv("maskrm", NRM_T)[:, 2 * h:2 * h + 2])
            muim = tl.tile([C, 2], F32, tag="muim")
            nc.gpsimd.tensor_mul(muim, mui2, fv("maskrm", NRM_T)[:, 2 * h:2 * h + 2])
            yield
            xhat2 = tl.tile([C, 2, C], F32, tag="xhat2")
            for t in range(2):
                if NORM_ACT:
                    nc.scalar.activation(xhat2[:, t], x2rm[:, t], AF.Identity,
                                         bias=muim[:, t:t + 1], scale=isdm[:, t:t + 1])
                else:
                    nc.vector.tensor_scalar(xhat2[:, t], x2rm[:, t],
                                            mv2[:, t, 0:1], isdm[:, t:t + 1],
                                            op0=OP.subtract, op1=OP.mult)
            yield
            # x3 = xhat2*gbc2 + bbm   (row-major; both ops on Pool)
            x3a = tl.tile([C, 2, C], F32, tag="x3a")
            for t in range(2):
                nc.gpsimd.tensor_mul(x3a[:, t], xhat2[:, t], fv(f"gbc2_{l}", C))
            x3 = tl.tile([C, 2, C], F32, tag="x3")
            for t in range(2):
                nc.gpsimd.tensor_add(x3[:, t], x3a[:, t], bbm[:, l, 2 * h + t])
            yield
            if last:
                for t in range(2):
                    nc.scalar.dma_start(
                        out_d.ap()[h * NHALF + t * C:h * NHALF + (t + 1) * C, :],
                        x3[:, t])
                return
            # transpose x3 back to channel-major -> xs[l+1]; compute pern[l+1]
            x3c_ps = r0.rearrange("p (a b) -> p a b", a=2)  # dd dead after x2 stt
            for t in range(2):
                nc.tensor.transpose(x3c_ps[:, t], x3[:, t], i128f.bitcast(F32))
            nc.vector.tensor_copy(xs_all[l + 1][:, nsl],
                                  x3c_ps.rearrange("p a b -> p (a b)"))
            yield
            pp = r1  # free after the LN2 normalize reads
            nc.tensor.matmul(pp, w1a[l + 1], xs_all[l + 1][:, nsl], start=True, stop=True)
            nc.vector.scalar_tensor_tensor(
                pern_all[l + 1][:, nsl], in0=pp, scalar=0.0,
                in1=n0pern[l + 1][:, nsl], op0=OP.bypass, op1=OP.add)

        # ---------------- main loop ----------------
        # Unified lag-2 pipeline over both halves: at step t the PE queue
        # gets mm1/pern for span t, gelu-A for span t-1, and mm2/gelu-B/msum
        # for span t-2.  The two-step lag means every PE consumer is emitted
        # a full span-time (~2.1us) after its gelu producer started, so a
        # gelu running long never stalls the in-order PE queue; t1 PSUM pool
        # holds 3 spans (6 banks) to cover the deeper pipeline.  Tails are
        # pumped from a deque between span steps, spilling into the next
        # half/layer's stream.
        from collections import deque
        tails = deque()

        def pump(k=1):
            for _ in range(k):
                if not tails:
                    return
                try:
                    next(tails[0])
                except StopIteration:
                    tails.popleft()

        NT = 2 * NSPAN
        for l in range(V_LAYERS):
            msumall = msump.tile([C, 2, NHALF], F32, tag="ms", name="msum")
            msums = [msumall[:, 0], msumall[:, 1]]
            states = [{}, {}]
            for t in range(NT + 2):
                if t < NT:
                    h, s = divmod(t, NSPAN)
                    if s == 0 and l > 0:
                        # the tail producing pern[l][:, h] must be fully
                        # emitted before any consumer of it is emitted
                        # (program order is dependency order for the tracker)
                        while len(tails) > (1 - h):
                            pump()
                    emit_stream_span(l, h, s, states[h])
                if 1 <= t < NT + 1:
                    ha, sa = divmod(t - 1, NSPAN)
                    emit_gelu_A(l, ha, sa, states[ha])
                if t >= 2:
                    hb, sb = divmod(t - 2, NSPAN)
                    emit_mm2(l, hb, sb, states[hb])
                    emit_gelu_B(l, hb, sb, states[hb])
                    emit_msum(l, hb, sb, states[hb], msums[hb])
                    if sb == NSPAN - 1:
                        tails.append(emit_tail(l, hb, msums[hb]))
                pump(2 if len(tails) > 1 else 1)
        while tails:
            pump()

    if os.environ.get("KV_DEDUP", "1") == "1":
        _dedup_ldweights(nc)
    nc.compile()
    return nc


def _dedup_ldweights(nc):
    """Drop InstLdweights that reload the PE stationary operand already in
    the array. tile_legalize emits one LDW per bf16 matmul even for runs of
    matmuls sharing one weight (mm1 x2, pern x2, msum x4); the redundant LDW
    serializes ~107ns each on the PE queue. fp32/fp32r matmuls (transposes,
    tail pern-next) are self-loading at walrus level and invalidate the
    loaded weight, as does any kept LDW with a different key."""
    removed = 0
    for blk in nc.main_func.blocks:
        last_key = None
        keep = []
        for inst in blk.instructions:
            tn = type(inst).__name__
            if tn == "InstLdweights":
                w = inst.ins[0]
                si = inst.sync_info
                clean = si is None or (not si.on_wait and not si.on_update)
                key = (str(w.memref), w.offset, str(w.ap), str(w.dtype),
                       str(inst.perf_mode), str(inst.is_transpose),
                       str(inst.tile_position))
                if clean and str(w.dtype) == "dt.bfloat16" and key == last_key:
                    removed += 1
                    continue          # weights already loaded; drop
                last_key = key
            elif tn == "InstMatmult":
                wd = str(inst.ins[-1].dtype)
                if wd not in ("dt.bfloat16",):
                    last_key = None   # walrus inserts its own LDW here
            keep.append(inst)
        blk.instructions[:] = keep
    return removed


def _prep_inputs(inputs):
    """Host-side: shard over nodes, relayout, fold weight-only arithmetic."""
    bfseg, fseg, rseg = _seg_layout()
    nf = np.asarray(inputs["node_features"], dtype=np.float32)
    ef = np.asarray(inputs["edge_features"], dtype=np.float32)
    mask = np.asarray(inputs["mask"], dtype=np.float32)
    w1 = np.asarray(inputs["msg_w1"], dtype=np.float32)
    w2 = np.asarray(inputs["msg_w2"], dtype=np.float32)
    w3 = np.asarray(inputs["msg_w3"], dtype=np.float32)
    b1 = np.asarray(inputs["msg_b1"], dtype=np.float32)
    b2 = np.asarray(inputs["msg_b2"], dtype=np.float32)
    b3 = np.asarray(inputs["msg_b3"], dtype=np.float32)
    dw1 = np.asarray(inputs["d_w1"], dtype=np.float32)
    db1 = np.asarray(inputs["d_b1"], dtype=np.float32)
    dw2 = np.asarray(inputs["d_w2"], dtype=np.float32)
    db2 = np.asarray(inputs["d_b2"], dtype=np.float32)
    g1 = np.asarray(inputs["ln1_g"], dtype=np.float32)
    bb1 = np.asarray(inputs["ln1_b"], dtype=np.float32)
    g2 = np.asarray(inputs["ln2_g"], dtype=np.float32)
    bb2 = np.asarray(inputs["ln2_b"], dtype=np.float32)

    w1a = w1[:, 0:C, :]
    w1b = w1[:, C:2 * C, :]
    w1d = w1[:, 3 * C:4 * C, :]
    w3e = w3 / SCALE
    b3e = b3 * (K / SCALE)
    # dense folds: input is xhat (normalized, no affine); LN1 g/b folded in.
    dw1f = g1[:, :, None] * dw1                     # [L, C, H]
    db1f = db1 + np.einsum("lc,lch->lh", bb1, dw1)  # [L, H]

    # ---- shared packed tensors ----
    packbf_shared = np.zeros((C, bfseg.cols), dtype=BF)
    for l in range(L):
        o = bfseg.off[f"w1d{l}"]
        packbf_shared[:, o:o + C] = w1d[l].astype(BF)
        o = bfseg.off[f"w2{l}"]
        packbf_shared[:, o:o + C] = w2[l].astype(BF)
        o = bfseg.off[f"w3e{l}"]
        packbf_shared[:, o:o + C] = w3e[l].astype(BF)
        for ch in range(HCH):
            o = bfseg.off[f"dw1_{l}_{ch}"]
            packbf_shared[:, o:o + C] = dw1f[l][:, ch * C:(ch + 1) * C].astype(BF)
            o = bfseg.off[f"dw2_{l}_{ch}"]
            packbf_shared[:, o:o + C] = dw2[l][ch * C:(ch + 1) * C, :].astype(BF)
    o = bfseg.off["i128bf"]
    packbf_shared[:, o:o + C] = np.eye(C, dtype=BF)

    packf_shared = np.zeros((C, fseg.cols), dtype=np.float32)
    packr_shared = np.zeros((C, rseg.cols), dtype=np.float32)

    def put(name, arr):
        o = fseg.off[name]
        packf_shared[:, o:o + arr.shape[1]] = arr

    def putr(name, arr):
        o = rseg.off[name]
        packr_shared[:, o:o + arr.shape[1]] = arr

    putr("w1a1", w1a[1])
    putr("w1a2", w1a[2])
    putr("i128f", np.eye(C, dtype=np.float32))
    for l in range(L):
        put(f"gbc2_{l}", np.broadcast_to(g2[l][None, :], (C, C)))
        put(f"bbc2_{l}", np.broadcast_to(bb2[l][None, :], (C, C)))
    put("b3e", b3e.T.copy())      # [C, L]
    put("db2", db2.T.copy())
    put("g1", g1.T.copy())
    put("b1", bb1.T.copy())
    put("b2", b2.T.copy())
    put("c12", (bb1 + db2).T.copy())
    put("db1", db1f.reshape(L * HCH, C).T.copy())
    packf_shared[:, fseg.off["eps"]] = EPS

    # host-computed per-node biases
    pern0_full = nf @ (w1a[0] + w1b[0]) + b1[0]          # [N, C]
    n0p1_full = nf @ w1b[1] + b1[1]
    n0p2_full = nf @ w1b[2] + b1[2]

    in_maps = []
    for core in range(NCORES):
        n0 = core * NLOC
        esh = ef[n0:n0 + NLOC]                       # [512, 48, 128]
        ekm = esh.transpose(2, 1, 0)                 # [128c, 48k, 512n]
        ekm = ekm.reshape(C, K, 2, NHALF).transpose(0, 2, 1, 3)
        ekm = np.ascontiguousarray(ekm.reshape(C, 2 * K * NHALF)).astype(BF)
        packf = packf_shared.copy()
        packbf = packbf_shared.copy()
        o = fseg.off["x0"]
        packf[:, o:o + NLOC] = nf[n0:n0 + NLOC].T
        o = bfseg.off["pern0"]
        packbf[:, o:o + NLOC] = pern0_full[n0:n0 + NLOC].T.astype(BF)
        o = fseg.off["n0pern1"]
        packf[:, o:o + NLOC] = n0p1_full[n0:n0 + NLOC].T
        o = fseg.off["n0pern2"]
        packf[:, o:o + NLOC] = n0p2_full[n0:n0 + NLOC].T
        o = fseg.off["maskrm"]
        packf[:, o:o + NRM_T] = mask[n0:n0 + NLOC].reshape(NRM_T, C).T
        in_maps.append({"edge_km": ekm, "packbf": packbf, "packf": packf,
                        "packr": packr_shared})
    return in_maps


def kernel(**inputs) -> np.ndarray:
    if "nc" not in _CACHED:
        _CACHED["nc"] = _build()
    nc = _CACHED["nc"]
    in_maps = _prep_inputs(inputs)
    res = run_bass_kernel_spmd(nc, in_maps, core_ids=list(range(NCORES)))
    out = np.concatenate([res.results[c]["out"] for c in range(NCORES)], axis=0)
    _CACHED["last_results"] = res
    return out

